# revision 6
# baseline (speedup 1.0000x reference)
"""Trainium2 Bass kernel for the channel-attention + BN + residual block.

Full inputs -> full output. Internally: 8 NeuronCores, core = 2*b + h
(batch b in [0,4), spatial half h in [0,2)).

Math restructure (algebraically exact):
  G  = x @ x^T  (per batch, [64,64] Gram), Sx = row sums
  f  = phi_w G th_w^T + (phi_w Sx) th_b^T + phi_b (th_w Sx)^T + N phi_b th_b^T
  attn = softmax(f)  (fp32 on device)
  A = attn @ g_w, cvec = attn @ g_b   =>  y = A x + cvec 1^T
  torch-permute becomes an index remap: w_y[o, j*64+c] = sum_c' w_w[o,c'] y[c, c'*1024+j] + ...
  BN stats via cross-core AllReduce, then normalize + residual.
"""

import numpy as np
import ml_dtypes
from contextlib import ExitStack

import concourse.bass as bass
import concourse.tile as tile
from concourse import mybir, bacc
from concourse.bass_utils import run_bass_kernel_spmd

F32 = mybir.dt.float32
BF16 = mybir.dt.bfloat16
AL = mybir.AluOpType
AF = mybir.ActivationFunctionType
AX = mybir.AxisListType

NCORES = 8
C = 64
N = 65536          # H*W
NH = N // 2        # per-core spatial columns (32768)
CNT = 4 * N        # BN count over (B,H,W)
BN_EPS = 1e-5
NQT = NH // 128    # 256 Y^T tiles of 128 q-rows
NT = NH // 1024    # 32 paired W/final tiles ([128,512] covering 1024 outputs)


def _build():
    nc = bacc.Bacc("TRN2", target_bir_lowering=False, debug=False, num_devices=NCORES)

    xs = nc.dram_tensor("xs", [C, NH], BF16, kind="ExternalInput").ap()
    xr = nc.dram_tensor("xr", [C, NH], F32, kind="ExternalInput").ap()
    wth = nc.dram_tensor("wth", [C + 1, C], F32, kind="ExternalInput").ap()     # [th_w^T; th_b]
    phwt = nc.dram_tensor("phwt", [C, C], F32, kind="ExternalInput").ap()       # phi_w^T
    thwt = nc.dram_tensor("thwt", [C, C], F32, kind="ExternalInput").ap()       # th_w^T
    gwl = nc.dram_tensor("gwl", [C, C], F32, kind="ExternalInput").ap()         # g_w (raw)
    ident = nc.dram_tensor("ident", [C, C], F32, kind="ExternalInput").ap()
    phib = nc.dram_tensor("phib", [1, C], F32, kind="ExternalInput").ap()       # phi_b row
    thbn = nc.dram_tensor("thbn", [1, C], F32, kind="ExternalInput").ap()       # N*th_b row
    gb = nc.dram_tensor("gb", [C, 1], F32, kind="ExternalInput").ap()           # g_b col
    ww2 = nc.dram_tensor("ww2", [128, 128], BF16, kind="ExternalInput").ap()    # blockdiag(w_w^T, w_w^T)
    add2 = nc.dram_tensor("add2", [2, 128], BF16, kind="ExternalInput").ap()    # [[sw|sw],[wb|wb]]
    crow = nc.dram_tensor("crow", [2, 512], BF16, kind="ExternalInput").ap()    # row1 = ones
    sel = nc.dram_tensor("sel", [128, C], F32, kind="ExternalInput").ap()       # fold selector
    gamma = nc.dram_tensor("gamma", [C, 1], F32, kind="ExternalInput").ap()
    beta = nc.dram_tensor("beta", [C, 1], F32, kind="ExternalInput").ap()

    out = nc.dram_tensor("out", [C, NH], F32, kind="ExternalOutput").ap()

    with tile.TileContext(nc) as tc, ExitStack() as ctx:
        big = ctx.enter_context(tc.tile_pool(name="big", bufs=1))
        wp = ctx.enter_context(tc.tile_pool(name="wp", bufs=1))
        st = ctx.enter_context(tc.tile_pool(name="st", bufs=2))
        fin = ctx.enter_context(tc.tile_pool(name="fin", bufs=3))
        pg = ctx.enter_context(tc.tile_pool(name="pg", bufs=1, space="PSUM"))
        py = ctx.enter_context(tc.tile_pool(name="py", bufs=2, space="PSUM"))
        pw = ctx.enter_context(tc.tile_pool(name="pw", bufs=2, space="PSUM"))
        pt = ctx.enter_context(tc.tile_pool(name="pt", bufs=2, space="PSUM"))
        dram = ctx.enter_context(tc.tile_pool(name="dram", bufs=1, space="DRAM"))

        # ---- small weight tiles ----
        def wtile(src, shape, dtype, tag):
            t = wp.tile(shape, dtype, tag=tag)
            nc.sync.dma_start(out=t[:], in_=src[:])
            return t

        wth_sb = wtile(wth, [C + 1, C], F32, "wth")
        phwt_sb = wtile(phwt, [C, C], F32, "phwt")
        thwt_sb = wtile(thwt, [C, C], F32, "thwt")
        gwl_sb = wtile(gwl, [C, C], F32, "gwl")
        id_sb = wtile(ident, [C, C], F32, "ident")
        phib_sb = wtile(phib, [1, C], F32, "phib")
        thbn_sb = wtile(thbn, [1, C], F32, "thbn")
        gb_sb = wtile(gb, [C, 1], F32, "gb")
        ww2_sb = wtile(ww2, [128, 128], BF16, "ww2")
        add2_sb = wtile(add2, [2, 128], BF16, "add2")
        const2 = wtile(crow, [2, 512], BF16, "crow")
        sel_sb = wtile(sel, [128, C], F32, "sel")
        gamma_sb = wtile(gamma, [C, 1], F32, "gamma")
        beta_sb = wtile(beta, [C, 1], F32, "beta")

        # ---- phase 1: load xs (bf16, resident) + row sums Sx ----
        xs_sb = big.tile([C, NH], BF16, tag="xs")
        sxp = wp.tile([C, 8], F32)
        for i in range(8):
            sl = slice(i * 4096, (i + 1) * 4096)
            nc.sync.dma_start(out=xs_sb[:, sl], in_=xs[:, sl])
            nc.vector.tensor_reduce(sxp[:, i : i + 1], xs_sb[:, sl], axis=AX.X, op=AL.add)
        sx = wp.tile([C, 1], F32)
        nc.vector.tensor_reduce(sx[:], sxp[:], axis=AX.X, op=AL.add)

        # ---- phase 1b: Gram matrix via xbar-transposed tiles ----
        g_ps = pg.tile([C, C], F32)
        for it in range(8):
            xtt = st.tile([128, 32, C], BF16, tag="xtt")
            nc.scalar.dma_start(out=xtt[:], in_=xs[:, it * 4096 : (it + 1) * 4096], transpose=True)
            for k in range(32):
                nc.tensor.matmul(
                    g_ps[:],
                    lhsT=xtt[:, k, :],
                    rhs=xtt[:, k, :],
                    start=(it == 0 and k == 0),
                    stop=(it == 7 and k == 31),
                )

        gsx = wp.tile([C, C + 1], F32)
        nc.vector.tensor_copy(out=gsx[:, 0:C], in_=g_ps[:])
        nc.vector.tensor_copy(out=gsx[:, C : C + 1], in_=sx[:])

        # ---- pairwise AllReduce of [G | Sx] ----
        cin = dram.tile([C, C + 1], F32)
        cout = dram.tile([C, C + 1], F32)
        nc.sync.dma_start(out=cin[:], in_=gsx[:])
        nc.gpsimd.collective_compute(
            "AllReduce", AL.add,
            replica_groups=[[0, 1], [2, 3], [4, 5], [6, 7]],
            ins=[cin[:].opt()], outs=[cout[:].opt()],
        )
        gsx2 = wp.tile([C, C + 1], F32)
        nc.sync.dma_start(out=gsx2[:], in_=cout[:])

        # ---- phase 2: f, softmax, A^T, cvec (all [64,64]-scale) ----
        gsxT_ps = pt.tile([128, C], F32, tag="tiny")
        nc.tensor.transpose(gsxT_ps[0 : C + 1, :], gsx2[:], id_sb[:])
        gsxT_sb = wp.tile([C + 1, C], F32)
        nc.vector.tensor_copy(out=gsxT_sb[:], in_=gsxT_ps[0 : C + 1, :])

        inner_ps = pt.tile([C, C], F32, tag="tiny")
        nc.tensor.matmul(inner_ps[:], lhsT=gsxT_sb[:], rhs=wth_sb[:], start=True, stop=True)
        inner_sb = wp.tile([C, C], F32)
        nc.vector.tensor_copy(out=inner_sb[:], in_=inner_ps[:])

        f_ps = pt.tile([C, C], F32, tag="tiny")
        nc.tensor.matmul(f_ps[:], lhsT=phwt_sb[:], rhs=inner_sb[:], start=True, stop=False)
        vrow_ps = pt.tile([1, C], F32, tag="tiny")
        nc.tensor.matmul(vrow_ps[:], lhsT=gsx2[:, C : C + 1], rhs=thwt_sb[:], start=True, stop=True)
        vrow_sb = wp.tile([1, C], F32)
        nc.vector.tensor_tensor(out=vrow_sb[:], in0=vrow_ps[:], in1=thbn_sb[:], op=AL.add)
        nc.tensor.matmul(f_ps[:], lhsT=phib_sb[:], rhs=vrow_sb[:], start=False, stop=True)

        f_sb = wp.tile([C, C], F32)
        nc.vector.tensor_copy(out=f_sb[:], in_=f_ps[:])
        negmax = wp.tile([C, 1], F32)
        nc.vector.tensor_reduce(negmax[:], f_sb[:], axis=AX.X, op=AL.max, negate=True)
        esum = wp.tile([C, 1], F32)
        attn = wp.tile([C, C], F32)
        nc.scalar.activation(out=attn[:], in_=f_sb[:], func=AF.Exp,
                             bias=negmax[:], scale=1.0, accum_out=esum[:])
        rinv = wp.tile([C, 1], F32)
        nc.vector.reciprocal(rinv[:], esum[:])
        nc.vector.tensor_scalar(out=attn[:], in0=attn[:], scalar1=rinv[:], scalar2=None, op0=AL.mult)

        attnT_ps = pt.tile([C, C], F32, tag="tiny")
        nc.tensor.transpose(attnT_ps[:], attn[:], id_sb[:])
        attnT_sb = wp.tile([C, C], F32)
        nc.vector.tensor_copy(out=attnT_sb[:], in_=attnT_ps[:])

        at_ps = pt.tile([C, C], F32, tag="tiny")
        nc.tensor.matmul(at_ps[:], lhsT=gwl_sb[:], rhs=attnT_sb[:], start=True, stop=True)
        atb = wp.tile([C, C], BF16)
        nc.vector.tensor_copy(out=atb[:], in_=at_ps[:])

        cvec_ps = pt.tile([C, 1], F32, tag="tiny")
        nc.tensor.matmul(cvec_ps[:], lhsT=attnT_sb[:], rhs=gb_sb[:], start=True, stop=True)
        cvec_sb = wp.tile([C, 1], F32)
        nc.vector.tensor_copy(out=cvec_sb[:], in_=cvec_ps[:])
        cvT_ps = pt.tile([1, C], F32, tag="tiny")
        nc.tensor.transpose(cvT_ps[:], cvec_sb[:], id_sb[:])
        cvr = wp.tile([1, C], F32)
        nc.vector.tensor_copy(out=cvr[:], in_=cvT_ps[:])
        for r in range(8):
            nc.vector.tensor_copy(out=const2[0:1, r * 64 : (r + 1) * 64], in_=cvr[:])

        # ---- phase 3: Y^T tiles ([128 q, 64 c]) ----
        # yt layout: [128 partitions, NQT*64 free]; tile qt occupies cols [64*qt, 64*qt+64)
        yt = big.tile([128, NQT * C], BF16, tag="yt2")
        for blk in range(NQT // 8):
            py_t = py.tile([128, 512], F32, tag="ypsum")
            for s in range(8):
                qt = blk * 8 + s
                nc.tensor.matmul(
                    py_t[:, s * C : (s + 1) * C],
                    lhsT=xs_sb[:, qt * 128 : (qt + 1) * 128],
                    rhs=atb[:],
                    start=True, stop=True,
                )
            dst = yt[:, blk * 512 : (blk + 1) * 512]
            if blk % 2 == 0:
                nc.scalar.activation(out=dst, in_=py_t[:], func=AF.Copy)
            else:
                nc.vector.tensor_copy(out=dst, in_=py_t[:])

        # ---- phase 4: T_s gather (SBUF->SBUF DMA, permuted) ----
        ts = big.tile([128, NT * 512], BF16, tag="ts")
        # views: yt free col = 256*cp + 64*tq + c ; ts free col = 4096*tq + 512*g + 64*jj8 + c
        yt_v = yt[:].rearrange("p (cp tq c) -> p cp tq c", cp=64, tq=4)
        ts_v = ts[:].rearrange("p (tq g jj c) -> p tq g jj c", tq=4, g=8, jj=8)
        for jj in range(16):
            half, jj8 = jj // 8, jj % 8
            eng = nc.sync if jj % 2 == 0 else nc.gpsimd
            for g in range(8):
                p = 16 * g + jj
                eng.dma_start(
                    out=ts_v[half * 64 : (half + 1) * 64, :, g, jj8, :],
                    in_=yt_v[p : p + 1, :, :, :],
                )

        # ---- phase 5: W matmuls + BN stats ----
        wy = big.tile([128, NT * 512], BF16, tag="wy")
        stats_all = wp.tile([128, NT, 6], F32)
        for t in range(NT):
            pw_t = pw.tile([128, 512], F32, tag="wpsum")
            nc.tensor.matmul(pw_t[:], lhsT=ww2_sb[:], rhs=ts[:, t * 512 : (t + 1) * 512],
                             start=True, stop=False)
            nc.tensor.matmul(pw_t[:], lhsT=add2_sb[:], rhs=const2[:], start=False, stop=True)
            nc.vector.bn_stats(out=stats_all[:, t, :], in_=pw_t[:])
            nc.scalar.activation(out=wy[:, t * 512 : (t + 1) * 512], in_=pw_t[:], func=AF.Copy)

        # ---- phase 6: BN stats fold + AllReduce + scale/shift ----
        mv = wp.tile([128, 2], F32)
        nc.vector.bn_aggr(out=mv[:], in_=stats_all[:])
        ss = wp.tile([128, 2], F32)
        nc.vector.tensor_scalar(out=ss[:, 0:1], in0=mv[:, 0:1], scalar1=float(NH // 2),
                                scalar2=None, op0=AL.mult)
        m2 = wp.tile([128, 1], F32)
        nc.vector.tensor_tensor(out=m2[:], in0=mv[:, 0:1], in1=mv[:, 0:1], op=AL.mult)
        nc.vector.tensor_tensor(out=m2[:], in0=m2[:], in1=mv[:, 1:2], op=AL.add)
        nc.vector.tensor_scalar(out=ss[:, 1:2], in0=m2[:], scalar1=float(NH // 2),
                                scalar2=None, op0=AL.mult)

        sums_ps = pt.tile([C, 2], F32, tag="tiny")
        nc.tensor.matmul(sums_ps[:], lhsT=sel_sb[:], rhs=ss[:], start=True, stop=True)
        sums_sb = wp.tile([C, 2], F32)
        nc.vector.tensor_copy(out=sums_sb[:], in_=sums_ps[:])

        cin2 = dram.tile([C, 2], F32)
        cout2 = dram.tile([C, 2], F32)
        nc.sync.dma_start(out=cin2[:], in_=sums_sb[:])
        nc.gpsimd.collective_compute(
            "AllReduce", AL.add,
            replica_groups=[list(range(NCORES))],
            ins=[cin2[:].opt()], outs=[cout2[:].opt()],
        )
        tot = wp.tile([C, 2], F32)
        nc.sync.dma_start(out=tot[:], in_=cout2[:])

        mean = wp.tile([C, 1], F32)
        nc.vector.tensor_scalar(out=mean[:], in0=tot[:, 0:1], scalar1=1.0 / CNT,
                                scalar2=None, op0=AL.mult)
        var = wp.tile([C, 1], F32)
        nc.vector.tensor_scalar(out=var[:], in0=tot[:, 1:2], scalar1=1.0 / CNT,
                                scalar2=None, op0=AL.mult)
        mm = wp.tile([C, 1], F32)
        nc.vector.tensor_tensor(out=mm[:], in0=mean[:], in1=mean[:], op=AL.mult)
        nc.vector.tensor_tensor(out=var[:], in0=var[:], in1=mm[:], op=AL.subtract)
        eps_t = wp.tile([C, 1], F32)
        nc.vector.memset(eps_t[:], BN_EPS)
        std = wp.tile([C, 1], F32)
        nc.scalar.activation(out=std[:], in_=var[:], func=AF.Sqrt, bias=eps_t[:], scale=1.0)
        rstd = wp.tile([C, 1], F32)
        nc.vector.reciprocal(rstd[:], std[:])
        a2sh = wp.tile([128, 2], F32)
        nc.vector.tensor_tensor(out=a2sh[0:C, 0:1], in0=rstd[:], in1=gamma_sb[:], op=AL.mult)
        nc.vector.tensor_tensor(out=mm[:], in0=mean[:], in1=a2sh[0:C, 0:1], op=AL.mult)
        nc.vector.tensor_tensor(out=a2sh[0:C, 1:2], in0=beta_sb[:], in1=mm[:], op=AL.subtract)
        nc.sync.dma_start(out=a2sh[C:128, :], in_=a2sh[0:C, :])

        # ---- phase 7: normalize + residual + store ----
        for t in range(NT):
            xrt = fin.tile([128, 512], F32, tag="xrt")
            nc.gpsimd.dma_start(out=xrt[0:C, :], in_=xr[:, t * 1024 : t * 1024 + 512])
            nc.gpsimd.dma_start(out=xrt[C:128, :], in_=xr[:, t * 1024 + 512 : (t + 1) * 1024])
            tmp = fin.tile([128, 512], F32, tag="tmp")
            nc.vector.tensor_scalar(out=tmp[:], in0=wy[:, t * 512 : (t + 1) * 512],
                                    scalar1=a2sh[:, 0:1], scalar2=a2sh[:, 1:2],
                                    op0=AL.mult, op1=AL.add)
            ot = fin.tile([128, 512], F32, tag="ot")
            nc.vector.tensor_tensor(out=ot[:], in0=tmp[:], in1=xrt[:], op=AL.add)
            nc.sync.dma_start(out=out[:, t * 1024 : t * 1024 + 512], in_=ot[0:C, :])
            nc.sync.dma_start(out=out[:, t * 1024 + 512 : (t + 1) * 1024], in_=ot[C:128, :])

    nc.compile()
    return nc


_NC_CACHE = None


def _get_nc():
    global _NC_CACHE
    if _NC_CACHE is None:
        _NC_CACHE = _build()
    return _NC_CACHE


def _host_prep(x, theta_w, theta_b, phi_w, phi_b, g_w, g_b, w_w, w_b, bn_gamma, bn_beta):
    bf = ml_dtypes.bfloat16
    shared = {
        "wth": np.concatenate([theta_w.T, theta_b[None, :]], axis=0).astype(np.float32),
        "phwt": np.ascontiguousarray(phi_w.T).astype(np.float32),
        "thwt": np.ascontiguousarray(theta_w.T).astype(np.float32),
        "gwl": np.ascontiguousarray(g_w).astype(np.float32),
        "ident": np.eye(C, dtype=np.float32),
        "phib": phi_b[None, :].astype(np.float32),
        "thbn": (float(N) * theta_b)[None, :].astype(np.float32),
        "gb": g_b[:, None].astype(np.float32),
        "gamma": bn_gamma[:, None].astype(np.float32),
        "beta": bn_beta[:, None].astype(np.float32),
    }
    wwT = w_w.T.astype(np.float32)
    ww2 = np.zeros((128, 128), np.float32)
    ww2[0:C, 0:C] = wwT
    ww2[C:128, C:128] = wwT
    shared["ww2"] = ww2.astype(bf)
    sw = w_w.sum(axis=1).astype(np.float32)
    add2 = np.zeros((2, 128), np.float32)
    add2[0, 0:C] = sw
    add2[0, C:128] = sw
    add2[1, 0:C] = w_b
    add2[1, C:128] = w_b
    shared["add2"] = add2.astype(bf)
    crow = np.zeros((2, 512), np.float32)
    crow[1, :] = 1.0
    shared["crow"] = crow.astype(bf)
    selm = np.zeros((128, C), np.float32)
    for p in range(128):
        selm[p, p % C] = 1.0
    shared["sel"] = selm

    in_maps = []
    for core in range(NCORES):
        b, h = core // 2, core % 2
        xb = np.asarray(x[b]).reshape(C, N)
        xs = np.ascontiguousarray(
            xb.reshape(C, C, 1024)[:, :, h * 512 : (h + 1) * 512].reshape(C, NH)
        ).astype(bf)
        xrr = np.ascontiguousarray(xb[:, h * NH : (h + 1) * NH]).astype(np.float32)
        m = dict(shared)
        m["xs"] = xs
        m["xr"] = xrr
        in_maps.append(m)
    return in_maps


def kernel(**inputs):
    x = np.asarray(inputs["x"], dtype=np.float32)
    B, Cc, H, W = x.shape
    nc = _get_nc()
    in_maps = _host_prep(
        x,
        np.asarray(inputs["theta_w"], np.float32), np.asarray(inputs["theta_b"], np.float32),
        np.asarray(inputs["phi_w"], np.float32), np.asarray(inputs["phi_b"], np.float32),
        np.asarray(inputs["g_w"], np.float32), np.asarray(inputs["g_b"], np.float32),
        np.asarray(inputs["w_w"], np.float32), np.asarray(inputs["w_b"], np.float32),
        np.asarray(inputs["bn_gamma"], np.float32), np.asarray(inputs["bn_beta"], np.float32),
    )
    res = run_bass_kernel_spmd(nc, in_maps, core_ids=list(range(NCORES)))
    outf = np.empty((B, Cc, N), np.float32)
    for core in range(NCORES):
        b, h = core // 2, core % 2
        outf[b][:, h * NH : (h + 1) * NH] = res.results[core]["out"]
    return outf.reshape(B, Cc, H, W)
